# revision 17
# baseline (speedup 1.0000x reference)
"""Trainium2 Bass kernel for the ragged triangular-GEMM decoder.

Computation (reference): out[b, i, :] = sum_{l<=i} x[b, l, :] @ W_i[l]
with x: [128, 12, 4096] fp32, W_i: [(i+1), 4096, 768] fp32, out: [128, 12, 768].

Decompose the work into 624 units (i, l, u): output layer i, source
layer l <= i, 96-wide output column group u (A=768 -> 8 groups). Each
unit is a [4096f x 96] GEMM contribution. Units are distributed over
8 cores x 5 "slots"; a slot is (one x source chunk l) x (a fixed-width
stack of units of that l). Slot unit-counts [24, 20, 16, 10, 8] sum to
78 = 624/8 and admit an EXACT partition of the ragged triangle (column
l has (12-l)*8 units), so the SPMD program is identical on every core
while each core reads only its 5 x-chunks (5.25 MB vs 25 MB full x)
and exactly 1/8 of all weights (61.4 MB fp16). Cores emit per-slot
partial sums; the host scatter-adds them into the final output.

Weight traffic dominates and sits at the HBM roofline; operands are
cast to fp16 on the host (halves HBM bytes, full-rate matmuls,
~3e-4 relative output error). PSUM accumulates in fp32.
"""

import numpy as np
from contextlib import ExitStack

import concourse.bass as bass
import concourse.tile as tile
from concourse import bacc, mybir
from concourse.bass_utils import run_bass_kernel_spmd

N_CORES = 8
B = 128
L = 12
F = 4096
A = 768
U = 96                      # unit width (output cols)
NU = A // U                 # 8 a-units per layer
KK = F // 128               # 32 k-chunks per source layer

SLOT_N = [32, 20, 18, 8]              # units per slot (sum 78)
SLOT_W = [n * U for n in SLOT_N]      # cols per slot
# slot processing order chosen so adjacent slots' PSUM chunks fit 8 banks
# (chunk counts: s0:6 s1:4 s2:4 s3:2)
SLOT_ORDER = [0, 3, 1, 2]

# source chunk l for (slot, core) -- the exact ragged-triangle partition
SLOT_L = [
    [0, 0, 0, 1, 3, 4, 4, 8],      # slot 0: 32 units each
    [1, 3, 3, 5, 6, 6, 7, 7],      # slot 1: 20
    [1, 1, 2, 2, 2, 2, 5, 5],      # slot 2: 18
    [2, 6, 9, 9, 9, 10, 10, 11],   # slot 3: 8
]

# kk-group sizing: keep each W DMA's per-partition chunk <= ~36 KB
_W_TILE_BYTES = 36000

_compiled_nc = None


def _unit_assignment():
    """-> units[(slot, core)] = list of (i, u), exactly SLOT_N[slot] long."""
    pieces_by_l = {l: [] for l in range(L)}
    for s in range(len(SLOT_N)):
        for c in range(N_CORES):
            pieces_by_l[SLOT_L[s][c]].append((s, c))
    out = {}
    for l in range(L):
        units = [(i, u) for i in range(l, L) for u in range(NU)]
        acc = 0
        for (s, c) in sorted(pieces_by_l[l]):
            n = SLOT_N[s]
            out[(s, c)] = units[acc:acc + n]
            acc += n
        assert acc == len(units), (l, acc, len(units))
    return out


def _kk_groups(w_cols: int, itemsize: int) -> list[tuple[int, int]]:
    bytes_per_kk = w_cols * itemsize
    n_groups = max(1, -(-KK * bytes_per_kk // _W_TILE_BYTES))
    kg = -(-KK // n_groups)
    out = []
    s = 0
    while s < KK:
        e = min(KK, s + kg)
        out.append((s, e))
        s = e
    return out


def _chunks(w: int) -> list[tuple[int, int]]:
    """Split w cols into <=512-wide PSUM-bank chunks."""
    out = []
    s = 0
    while s < w:
        out.append((s, min(w, s + 512)))
        s += 512
    return out


def _build():
    nc = bacc.Bacc("TRN2", target_bir_lowering=False, debug=False,
                   num_devices=N_CORES)

    rdt = mybir.dt.float16
    isz = 2
    xs_d = [nc.dram_tensor(f"xs{s}", [128, KK, B], rdt,
                           kind="ExternalInput").ap()
            for s in range(len(SLOT_N))]
    w_d = [nc.dram_tensor(f"w{s}", [128, KK, SLOT_W[s]], rdt,
                          kind="ExternalInput").ap()
           for s in range(len(SLOT_N))]
    out_d = [nc.dram_tensor(f"out{s}", [B, SLOT_W[s]], mybir.dt.float16,
                            kind="ExternalOutput").ap()
             for s in range(len(SLOT_N))]

    rings = [nc.sync, nc.scalar]
    ring_i = 0

    def next_ring():
        nonlocal ring_i
        r = rings[ring_i % 2]
        ring_i += 1
        return r

    with tile.TileContext(nc) as tc:
        with ExitStack() as ctx:
            xpool = ctx.enter_context(tc.tile_pool(name="x", bufs=3))
            wpool = ctx.enter_context(tc.tile_pool(name="w", bufs=3))
            opool = ctx.enter_context(tc.tile_pool(name="o", bufs=2))
            ppool = ctx.enter_context(tc.tile_pool(name="ps", bufs=8,
                                                   space="PSUM"))

            for si, s in enumerate(SLOT_ORDER):
                w_cols = SLOT_W[s]
                xl = xpool.tile([128, KK, B], rdt, tag="xl", name=f"x{s}")
                next_ring().dma_start(xl[:], xs_d[s][:])

                pcs = [ppool.tile([B, c1 - c0], mybir.dt.float32, tag="pc",
                                  name=f"pc{s}_{ci}")
                       for ci, (c0, c1) in enumerate(_chunks(w_cols))]

                for (g0, g1) in _kk_groups(w_cols, isz):
                    wg = wpool.tile([128, g1 - g0, w_cols], rdt, tag="wg",
                                    name=f"wg{s}_{g0}")
                    next_ring().dma_start(wg[:], w_d[s][:, g0:g1, :])

                    for kk in range(g0, g1):
                        for ci, (c0, c1) in enumerate(_chunks(w_cols)):
                            nc.tensor.matmul(
                                pcs[ci][:],
                                xl[:, kk, :],
                                wg[:, kk - g0, c0:c1],
                                start=(kk == 0), stop=(kk == KK - 1),
                            )

                ot = opool.tile([B, w_cols], mybir.dt.float16, tag="ot",
                                name=f"ot{s}")
                for ci, (c0, c1) in enumerate(_chunks(w_cols)):
                    nc.vector.tensor_copy(ot[:, c0:c1], pcs[ci][:])
                nc.sync.dma_start(out_d[s][:], ot[:])

    nc.compile()
    return nc


def _pack_inputs(x: np.ndarray, Ws: list[np.ndarray]):
    """Host-side shard + layout. Returns (in_maps, assignment)."""
    assign = _unit_assignment()
    # x chunks: xc[l][p, kk, b] = x[b, l, kk*128 + p], fp16
    xc = np.ascontiguousarray(
        x.reshape(B, L, KK, 128).transpose(1, 3, 2, 0)).astype(np.float16)
    # W lookup: wt[i][l] -> [128p, KK, A] fp16 view-ish (lazy per use)
    in_maps = []
    for c in range(N_CORES):
        m = {}
        for s in range(len(SLOT_N)):
            l = SLOT_L[s][c]
            m[f"xs{s}"] = xc[l]
            parts = []
            for (i, u) in assign[(s, c)]:
                wl = Ws[i][l]  # [F, A] fp32
                blk = wl.reshape(KK, 128, A)[:, :, u * U:(u + 1) * U]
                parts.append(blk.transpose(1, 0, 2))  # [128, KK, 96]
            m[f"w{s}"] = np.ascontiguousarray(
                np.concatenate(parts, axis=2)).astype(np.float16)
        in_maps.append(m)
    return in_maps, assign


def _run(inputs: dict, trace: bool = False):
    global _compiled_nc
    if _compiled_nc is None:
        _compiled_nc = _build()
    x = np.asarray(inputs["x"], dtype=np.float32)
    Ws = [np.asarray(inputs[f"W_{i}"], dtype=np.float32) for i in range(L)]
    in_maps, assign = _pack_inputs(x, Ws)
    res = run_bass_kernel_spmd(_compiled_nc, in_maps,
                               core_ids=list(range(N_CORES)), trace=trace)
    out = np.zeros((B, L, A), dtype=np.float32)
    for c in range(N_CORES):
        for s in range(len(SLOT_N)):
            part = res.results[c][f"out{s}"].astype(np.float32)
            for k, (i, u) in enumerate(assign[(s, c)]):
                out[:, i, u * U:(u + 1) * U] += part[:, k * U:(k + 1) * U]
    return out, res


def kernel(**inputs: np.ndarray) -> np.ndarray:
    out, _ = _run(inputs, trace=False)
    return out
